# Initial kernel scaffold
#
"""GCN actor (2x GCNConv + linear heads) on 8 Trainium2 NeuronCores.

Strategy (dst-sharded graph parallel):
  - Nodes row-sharded 8 ways. Weights replicated.
  - Per layer: z = dinv * (h @ W) computed locally per shard (TensorE),
    cast fp16, AllGathered into a replicated HBM table.
  - Aggregation per core over its in-edges (dst-owned, dst-sorted):
    transpose-mode dma_gather pulls source feature columns from the table
    (features land on partitions), VectorE does exact segmented reduction
    over uniform-degree runs, partial sums are PE-transposed to rows and
    CCE dma_scatter_add'ed into a per-shard fp32 accumulator in HBM.
  - Self loops are folded in analytically: out = dinv*(acc + z) + b, since
    the GCN norm dinv[s]*dinv[d] factorizes.
  - One Bass program serves all 8 cores (SPMD): the run schedule is padded
    to the max group-count over cores per (tile, chunk, degree-bucket);
    per-core variation lives in index/data inputs only.

Gather indices are int16, so the table is addressed in 4 chunks of 32768
rows; each (node, chunk) group reduces to one partial, combined by the
scatter-add.
"""

import os
import sys

for _p in ("/opt/trn_rl_repo", "/root/.axon_site/_ro/trn_rl_repo"):
    if os.path.isdir(_p) and _p not in sys.path:
        sys.path.insert(0, _p)

import numpy as np

import concourse.bacc as bacc
import concourse.bass as bass
import concourse.mybir as mybir
import concourse.tile as tile
from concourse.bass_utils import run_bass_kernel_spmd
from concourse.masks import make_identity

F = 128  # feature dim (fixed by problem)
NCORES = 8
CH = 32768  # rows addressable per gather call (int16 indices)
DCAP = 64  # max segment length per reduce group
# degree buckets: exact for small degrees, coarser above
DBUCKETS = list(range(1, 9)) + [10, 12, 14, 16, 20, 24, 32, 48, 64]
_BUCKET_LUT = np.zeros(DCAP + 1, dtype=np.int64)
for _d in range(1, DCAP + 1):
    _BUCKET_LUT[_d] = min(b for b in DBUCKETS if b >= _d)

f32 = mybir.dt.float32
f16 = mybir.dt.float16
i16 = mybir.dt.int16


def _wrap16(flat, ncols):
    """Wrap a flat int16 index stream into the [128, ncols] layout the Q7
    gather/scatter ucode expects: slot i at [i % 16, i // 16], replicated
    across the eight 16-partition core groups."""
    n = flat.shape[0]
    assert n % 16 == 0 and n // 16 <= ncols
    a = np.full((16, ncols), -1, dtype=np.int16)
    a[:, : n // 16] = flat.reshape(n // 16, 16).T
    return np.tile(a, (8, 1))


class Schedule:
    """Uniform (cross-core) aggregation schedule + per-core index data."""

    def __init__(self, n_nodes, tile_nodes):
        self.N = n_nodes
        self.NS = n_nodes // NCORES
        self.NT = -(-self.NS // 128)  # node row-tiles per core
        self.NSP = self.NT * 128  # padded shard rows
        self.NROWS = self.NSP * NCORES  # gather table rows
        self.NCH = -(-self.NROWS // CH)  # chunks
        self.TSN = tile_nodes
        self.n_tiles = -(-self.NSP // tile_nodes)
        # filled by build():
        self.runs = None  # [t][k] -> list of (bucket, m)
        self.S = None  # [t][k] -> padded gather slots
        self.gcol0 = None  # [t][k] -> gidx col offset
        self.P = None  # [t] -> partial count (scatter num_idxs)
        self.scol0 = None  # [t] -> sidx col offset
        self.GCOLS = 0
        self.SCOLS = 0
        self.gidx = None  # per-core [128, GCOLS] int16
        self.sidx = None  # per-core [128, SCOLS] int16


def build_schedule(src, dst, n_nodes, tile_nodes=1536):
    """Host preprocessing: group edges by (dst-core, dst-node, src-chunk),
    bucket group sizes, take the max group count over cores per
    (tile, chunk, bucket) as the shared quota, and serialize per-core
    gather/scatter index streams."""
    sch = Schedule(n_nodes, tile_nodes)
    NS, NSP, NCH, TSN = sch.NS, sch.NSP, sch.NCH, sch.TSN
    n_tiles = sch.n_tiles

    src = np.asarray(src, dtype=np.int64)
    dst = np.asarray(dst, dtype=np.int64)
    src_row = (src // NS) * NSP + (src % NS)  # table row (padded layout)
    core = dst // NS
    nloc = dst % NS
    chunk = src_row // CH

    # per-core grouped edges
    per_core = []  # (nloc_g, chunk_g, len_g, tile_g, bucket_g, srcs_sorted)
    for c in range(NCORES):
        m = core == c
        nl, ck, sr = nloc[m], chunk[m], src_row[m]
        order = np.lexsort((sr, ck, nl))
        nl, ck, sr = nl[order], ck[order], sr[order]
        if nl.size == 0:
            per_core.append(
                (np.zeros(0, np.int64),) * 5 + (np.zeros(0, np.int64),)
            )
            continue
        newg = np.empty(nl.size, dtype=bool)
        newg[0] = True
        newg[1:] = (nl[1:] != nl[:-1]) | (ck[1:] != ck[:-1])
        starts = np.flatnonzero(newg)
        lens = np.diff(np.append(starts, nl.size))
        g_nl, g_ck = nl[starts], ck[starts]
        # split groups longer than DCAP
        big = lens > DCAP
        if big.any():
            ns, nl2, nc2, nln = [], [], [], []
            for s, l, a, b in zip(starts[big], lens[big], g_nl[big], g_ck[big]):
                off = 0
                while off < l:
                    take = min(DCAP, l - off)
                    ns.append(s + off)
                    nln.append(take)
                    nl2.append(a)
                    nc2.append(b)
                    off += take
            starts = np.concatenate([starts[~big], np.array(ns, np.int64)])
            lens = np.concatenate([lens[~big], np.array(nln, np.int64)])
            g_nl = np.concatenate([g_nl[~big], np.array(nl2, np.int64)])
            g_ck = np.concatenate([g_ck[~big], np.array(nc2, np.int64)])
        g_tile = g_nl // TSN
        g_bucket = _BUCKET_LUT[lens]
        per_core.append((g_nl, g_ck, lens, g_tile, g_bucket, sr, starts))

    # quotas: max #groups over cores per (tile, chunk, bucket)
    quota = {}
    for c in range(NCORES):
        g_nl, g_ck, lens, g_tile, g_bucket, sr, starts = per_core[c]
        key = (g_tile * NCH + g_ck) * (DCAP + 1) + g_bucket
        uk, cnt = np.unique(key, return_counts=True)
        for k, n in zip(uk, cnt):
            quota[int(k)] = max(quota.get(int(k), 0), int(n))

    runs = [[[] for _ in range(NCH)] for _ in range(n_tiles)]
    for k, n in sorted(quota.items()):
        b = k % (DCAP + 1)
        tk = k // (DCAP + 1)
        t, ck = tk // NCH, tk % NCH
        runs[t][ck].append((int(b), int(n)))
    for t in range(n_tiles):
        for ck in range(NCH):
            runs[t][ck].sort(key=lambda x: -x[0])  # big buckets first

    S = [[0] * NCH for _ in range(n_tiles)]
    gcol0 = [[0] * NCH for _ in range(n_tiles)]
    P = [0] * n_tiles
    scol0 = [0] * n_tiles
    gc = 0
    sc = 0
    for t in range(n_tiles):
        p = 0
        for ck in range(NCH):
            s = sum(b * m for b, m in runs[t][ck])
            s = -(-s // 128) * 128
            S[t][ck] = s
            gcol0[t][ck] = gc
            gc += s // 16
            p += sum(m for _, m in runs[t][ck])
        P[t] = p
        scol0[t] = sc
        sc += -(-p // 16)
    sch.runs, sch.S, sch.gcol0, sch.P, sch.scol0 = runs, S, gcol0, P, scol0
    sch.GCOLS, sch.SCOLS = max(gc, 16), max(sc, 16)

    # serialize per-core index streams
    sch.gidx, sch.sidx = [], []
    for c in range(NCORES):
        g_nl, g_ck, lens, g_tile, g_bucket, sr, starts = per_core[c]
        key = ((g_tile * NCH + g_ck) * (DCAP + 1) + g_bucket).astype(np.int64)
        order = np.argsort(key, kind="stable")
        # group boundaries per (t, ck, b) in sorted order
        k_sorted = key[order]
        gi = np.zeros((128, sch.GCOLS), dtype=np.int16)
        si = np.full((128, sch.SCOLS), -1, dtype=np.int16)
        pos = 0  # cursor into `order`
        for t in range(n_tiles):
            sflat = []
            for ck in range(NCH):
                flat = []
                for b, mq in runs[t][ck]:
                    kk = (t * NCH + ck) * (DCAP + 1) + b
                    lo = np.searchsorted(k_sorted, kk, "left")
                    hi = np.searchsorted(k_sorted, kk, "right")
                    mem = order[lo:hi]
                    mreal = hi - lo
                    assert mreal <= mq
                    mat = np.zeros((mq, b), dtype=np.int16)
                    if mreal:
                        l_g = lens[mem]
                        rows = np.repeat(np.arange(mreal), l_g)
                        cols = np.arange(l_g.sum()) - np.repeat(
                            np.cumsum(l_g) - l_g, l_g
                        )
                        take = np.repeat(starts[mem], l_g) + cols
                        mat[rows, cols] = (sr[take] - g_ck[mem][0] * CH).astype(
                            np.int16
                        )
                    flat.append(mat.reshape(-1))
                    srow = np.full(mq, sch.NSP, dtype=np.int16)  # dump row
                    if mreal:
                        srow[:mreal] = g_nl[mem].astype(np.int16)
                    sflat.append(srow)
                flat = (
                    np.concatenate(flat) if flat else np.zeros(0, np.int16)
                )
                pad = S[t][ck] - flat.shape[0]
                flat = np.concatenate([flat, np.zeros(pad, np.int16)])
                nc_ = S[t][ck] // 16
                gi[:, gcol0[t][ck] : gcol0[t][ck] + nc_] = _wrap16(flat, nc_)
            sflat = (
                np.concatenate(sflat) if sflat else np.zeros(0, np.int16)
            )
            pad = -(-P[t] // 16) * 16 - sflat.shape[0]
            sflat = np.concatenate([sflat, np.full(pad, -1, np.int16)])
            ncs = -(-P[t] // 16)
            si[:, scol0[t] : scol0[t] + ncs] = _wrap16(sflat, ncs)
        sch.gidx.append(gi)
        sch.sidx.append(si)
    return sch


def build_bass(sch):
    """Build the single SPMD Bass program."""
    NT, NSP, NROWS, NCH = sch.NT, sch.NSP, sch.NROWS, sch.NCH
    n_tiles = sch.n_tiles
    NSPD = NSP + 128  # accumulator rows (incl. dump row at NSP)
    SMAX = max(max(r) for r in sch.S)
    PMAXB = max(-(-p // 128) for p in sch.P)  # partial row-blocks per tile

    nc = bacc.Bacc(
        "TRN2",
        target_bir_lowering=False,
        debug=False,
        enable_asserts=False,
        num_devices=NCORES,
    )

    # I/O
    x_in = nc.dram_tensor("x", [NSP, F], f32, kind="ExternalInput").ap()
    gidx_in = nc.dram_tensor(
        "gidx", [128, sch.GCOLS], i16, kind="ExternalInput"
    ).ap()
    sidx_in = nc.dram_tensor(
        "sidx", [128, sch.SCOLS], i16, kind="ExternalInput"
    ).ap()
    dinv_in = nc.dram_tensor("dinv", [128, NT], f32, kind="ExternalInput").ap()
    w1_in = nc.dram_tensor("w1", [F, F], f32, kind="ExternalInput").ap()
    w2_in = nc.dram_tensor("w2", [F, F], f32, kind="ExternalInput").ap()
    wh_in = nc.dram_tensor("wh", [F, 32], f32, kind="ExternalInput").ap()
    b1_in = nc.dram_tensor("b1r", [128, F], f32, kind="ExternalInput").ap()
    b2_in = nc.dram_tensor("b2r", [128, F], f32, kind="ExternalInput").ap()
    bh_in = nc.dram_tensor("bhr", [128, 32], f32, kind="ExternalInput").ap()
    zeros_in = nc.dram_tensor(
        "zeros", [NSPD, F], f32, kind="ExternalInput"
    ).ap()
    out_dram = nc.dram_tensor(
        "out", [NSP, 32], f32, kind="ExternalOutput"
    ).ap()

    # internal DRAM
    z_loc = [
        nc.dram_tensor(f"z_loc{i}", [NSP, F], f16).ap() for i in range(2)
    ]
    z_tab = [
        nc.dram_tensor(f"z_tab{i}", [NROWS, F], f16, addr_space="Shared").ap()
        for i in range(2)
    ]
    acc = [nc.dram_tensor(f"acc{i}", [NSPD, F], f32).ap() for i in range(2)]

    rg = [list(range(NCORES))]

    with tile.TileContext(nc) as tc:
        with (
            tc.tile_pool(name="const", bufs=1) as constp,
            tc.tile_pool(name="big", bufs=1) as bigp,
            tc.tile_pool(name="msg", bufs=2) as msgp,
            tc.tile_pool(name="arena", bufs=2) as arenap,
            tc.tile_pool(name="prow", bufs=2) as prowp,
            tc.tile_pool(name="gix", bufs=3) as gixp,
            tc.tile_pool(name="row", bufs=4) as rowp,
            tc.tile_pool(name="psum", bufs=4, space="PSUM") as psump,
            tc.tile_pool(name="psz", bufs=2, space="PSUM") as pszp,
        ):
            ident = constp.tile([128, 128], f32, tag="ident")
            make_identity(nc, ident[:])
            w1 = constp.tile([F, F], f32, tag="w1")
            nc.sync.dma_start(w1[:], w1_in)
            w2 = constp.tile([F, F], f32, tag="w2")
            nc.sync.dma_start(w2[:], w2_in)
            wh = constp.tile([F, 32], f32, tag="wh")
            nc.sync.dma_start(wh[:], wh_in)
            b1r = constp.tile([128, F], f32, tag="b1r")
            nc.sync.dma_start(b1r[:], b1_in)
            b2r = constp.tile([128, F], f32, tag="b2r")
            nc.sync.dma_start(b2r[:], b2_in)
            bhr = constp.tile([128, 32], f32, tag="bhr")
            nc.sync.dma_start(bhr[:], bh_in)
            dinv = constp.tile([128, NT], f32, tag="dinv")
            nc.sync.dma_start(dinv[:], dinv_in)

            # persistent [F, NSP] transposed activations (xT, then h1T, h2T)
            hT = bigp.tile([128, NSP], f32, tag="hT")

            def load_xT():
                for j in range(NT):
                    r = rowp.tile([128, F], f32, tag="ldrow")
                    nc.sync.dma_start(r[:], x_in[j * 128 : (j + 1) * 128, :])
                    ps = psump.tile([128, 128], f32, tag="tp")
                    nc.tensor.transpose(ps[:], r[:], ident[:])
                    nc.scalar.copy(hT[:, j * 128 : (j + 1) * 128], ps[:])

            def z_phase(li, w):
                """z = dinv * (h @ W) from hT; write fp16 rows to z_loc."""
                for j in range(NT):
                    ps = pszp.tile([128, F], f32, tag="zp")
                    nc.tensor.matmul(
                        ps[:],
                        lhsT=hT[:, j * 128 : (j + 1) * 128],
                        rhs=w[:],
                        start=True,
                        stop=True,
                    )
                    zr = rowp.tile([128, F], f16, tag="zrow")
                    nc.scalar.activation(
                        zr[:],
                        ps[:],
                        mybir.ActivationFunctionType.Copy,
                        scale=dinv[:, j : j + 1],
                    )
                    nc.sync.dma_start(
                        z_loc[li][j * 128 : (j + 1) * 128, :], zr[:]
                    )
                # init accumulator with zeros, gather table via AllGather
                nc.sync.dma_start(acc[li][:, :], zeros_in)
                nc.gpsimd.collective_compute(
                    "AllGather",
                    mybir.AluOpType.bypass,
                    replica_groups=rg,
                    ins=[z_loc[li][:, :]],
                    outs=[z_tab[li][:, :]],
                )

            def agg_phase(li):
                for t in range(n_tiles):
                    pblk = -(-sch.P[t] // 128)
                    arena = arenap.tile([128, PMAXB * 128], f32, tag="ar")
                    po = 0
                    for ck in range(NCH):
                        S = sch.S[t][ck]
                        if S == 0:
                            continue
                        gx = gixp.tile([128, SMAX // 16], i16, tag="gx")
                        c0 = sch.gcol0[t][ck]
                        nc.sync.dma_start(
                            gx[:, : S // 16], gidx_in[:, c0 : c0 + S // 16]
                        )
                        msg = msgp.tile([128, SMAX], f16, tag="msg")
                        rows = min(CH, NROWS - ck * CH)
                        nc.gpsimd.dma_gather(
                            out_ap=msg[:, :S].rearrange(
                                "p (a s) -> p a s", a=1
                            ),
                            in_ap=z_tab[li][ck * CH : ck * CH + rows, :],
                            idxs_ap=gx[:, : S // 16],
                            num_idxs=S,
                            num_idxs_reg=S,
                            elem_size=F,
                            transpose=True,
                        )
                        off = 0
                        for b, m in sch.runs[t][ck]:
                            nc.vector.tensor_reduce(
                                out=arena[:, po : po + m],
                                in_=msg[:, off : off + m * b].rearrange(
                                    "p (m b) -> p m b", b=b
                                ),
                                axis=mybir.AxisListType.X,
                                op=mybir.AluOpType.add,
                            )
                            po += m
                            off += m * b
                    # transpose partials to rows, scatter-add into acc
                    pr = prowp.tile([128, PMAXB, 128], f32, tag="pr")
                    for bblk in range(pblk):
                        ps = psump.tile([128, 128], f32, tag="tp")
                        nc.tensor.transpose(
                            ps[:],
                            arena[:, bblk * 128 : (bblk + 1) * 128],
                            ident[:],
                        )
                        nc.scalar.copy(pr[:, bblk, :], ps[:])
                    sx = gixp.tile([128, sch.SCOLS], i16, tag="sx")
                    ncs = -(-sch.P[t] // 16)
                    s0 = sch.scol0[t]
                    nc.sync.dma_start(
                        sx[:, :ncs], sidx_in[:, s0 : s0 + ncs]
                    )
                    nc.gpsimd.dma_scatter_add(
                        out_ap=acc[li][:, :],
                        in_ap=pr[:, :pblk, :],
                        idxs_ap=sx[:, :ncs],
                        num_idxs=sch.P[t],
                        num_idxs_reg=sch.P[t],
                        elem_size=F,
                    )

            def finish_layer(li, brep):
                """h = relu(dinv*(acc+z)+b); write hT in place."""
                for j in range(NT):
                    at = rowp.tile([128, F], f32, tag="accrow")
                    nc.sync.dma_start(
                        at[:], acc[li][j * 128 : (j + 1) * 128, :]
                    )
                    zt = rowp.tile([128, F], f16, tag="zrd")
                    nc.sync.dma_start(
                        zt[:], z_loc[li][j * 128 : (j + 1) * 128, :]
                    )
                    nc.vector.tensor_tensor(
                        out=at[:], in0=at[:], in1=zt[:], op=mybir.AluOpType.add
                    )
                    nc.vector.tensor_scalar_mul(at[:], at[:], dinv[:, j : j + 1])
                    nc.vector.tensor_tensor(
                        out=at[:], in0=at[:], in1=brep[:], op=mybir.AluOpType.add
                    )
                    hr = rowp.tile([128, F], f32, tag="hrow")
                    nc.scalar.activation(
                        hr[:], at[:], mybir.ActivationFunctionType.Relu
                    )
                    ps = psump.tile([128, 128], f32, tag="tp")
                    nc.tensor.transpose(ps[:], hr[:], ident[:])
                    nc.scalar.copy(hT[:, j * 128 : (j + 1) * 128], ps[:])

            def heads():
                for j in range(NT):
                    ps = pszp.tile([128, 32], f32, tag="hp")
                    nc.tensor.matmul(
                        ps[:],
                        lhsT=hT[:, j * 128 : (j + 1) * 128],
                        rhs=wh[:],
                        start=True,
                        stop=True,
                    )
                    ot = rowp.tile([128, 32], f32, tag="orow")
                    nc.vector.tensor_tensor(
                        out=ot[:], in0=ps[:], in1=bhr[:], op=mybir.AluOpType.add
                    )
                    nc.sync.dma_start(
                        out_dram[j * 128 : (j + 1) * 128, :], ot[:]
                    )

            load_xT()
            z_phase(0, w1)
            agg_phase(0)
            finish_layer(0, b1r)
            z_phase(1, w2)
            agg_phase(1)
            finish_layer(1, b2r)
            heads()

    nc.compile()
    return nc


def host_preprocess(inputs, n_nodes, tile_nodes=1536):
    x = np.asarray(inputs["x"], dtype=np.float32)
    ei = np.asarray(inputs["edge_index"])
    src, dst = ei[0].astype(np.int64), ei[1].astype(np.int64)

    deg = (np.bincount(dst, minlength=n_nodes) + 1).astype(np.float32)
    dinv = (1.0 / np.sqrt(deg)).astype(np.float32)

    sch = build_schedule(src, dst, n_nodes, tile_nodes)
    NS, NSP, NT = sch.NS, sch.NSP, sch.NT

    wh = np.concatenate(
        [np.asarray(inputs["Wm"], np.float32), np.asarray(inputs["Ws"], np.float32)],
        axis=1,
    )
    bh = np.concatenate(
        [np.asarray(inputs["bm"], np.float32), np.asarray(inputs["bs"], np.float32)]
    )
    b1 = np.asarray(inputs["b1"], np.float32)
    b2 = np.asarray(inputs["b2"], np.float32)

    in_maps = []
    for c in range(NCORES):
        xs = np.zeros((NSP, F), np.float32)
        xs[:NS] = x[c * NS : (c + 1) * NS]
        dv = np.ones(NSP, np.float32)
        dv[:NS] = dinv[c * NS : (c + 1) * NS]
        in_maps.append(
            {
                "x": xs,
                "gidx": sch.gidx[c],
                "sidx": sch.sidx[c],
                "dinv": dv.reshape(NT, 128).T.copy(),
                "w1": np.asarray(inputs["W1"], np.float32),
                "w2": np.asarray(inputs["W2"], np.float32),
                "wh": wh,
                "b1r": np.tile(b1[None, :], (128, 1)),
                "b2r": np.tile(b2[None, :], (128, 1)),
                "bhr": np.tile(bh[None, :], (128, 1)),
                "zeros": np.zeros((NSP + 128, F), np.float32),
            }
        )
    return sch, in_maps


def run_gcn(inputs, n_nodes, tile_nodes=1536, trace=False, **run_kwargs):
    sch, in_maps = host_preprocess(inputs, n_nodes, tile_nodes)
    nc = build_bass(sch)
    res = run_bass_kernel_spmd(
        nc, in_maps, list(range(NCORES)), trace=trace, **run_kwargs
    )
    NS = sch.NS
    outs = [np.asarray(res.results[c]["out"])[:NS] for c in range(NCORES)]
    full = np.concatenate(outs, axis=0)
    mean = np.ascontiguousarray(full[:, :16])
    logstd = np.ascontiguousarray(full[:, 16:])
    return (mean, logstd), res


def kernel(**inputs):
    (mean, logstd), _ = run_gcn(inputs, n_nodes=100000)
    return mean, logstd


# revision 47
# speedup vs baseline: 23.3396x; 23.3396x over previous
"""GCN actor (2x GCNConv + linear heads) on 8 Trainium2 NeuronCores.

Strategy (dst-sharded graph parallel):
  - Nodes row-sharded 8 ways. Weights replicated.
  - Per layer: z = dinv * (h @ W) computed locally per shard (TensorE),
    cast fp16, AllGathered into a replicated HBM table.
  - Aggregation per core over its in-edges (dst-owned, dst-sorted):
    transpose-mode dma_gather pulls source feature columns from the table
    (features land on partitions), VectorE does exact segmented reduction
    over uniform-degree runs, partial sums are PE-transposed to rows and
    CCE dma_scatter_add'ed into a per-shard fp32 accumulator in HBM.
  - Self loops are folded in analytically: out = dinv*(acc + z) + b, since
    the GCN norm dinv[s]*dinv[d] factorizes.
  - One Bass program serves all 8 cores (SPMD): the run schedule is padded
    to the max group-count over cores per (tile, chunk, degree-bucket);
    per-core variation lives in index/data inputs only.

Gather indices are int16, so the table is addressed in 4 chunks of 32768
rows; each (node, chunk) group reduces to one partial, combined by the
scatter-add.
"""

import os
import sys

for _p in ("/opt/trn_rl_repo", "/root/.axon_site/_ro/trn_rl_repo"):
    if os.path.isdir(_p) and _p not in sys.path:
        sys.path.insert(0, _p)

import numpy as np

import concourse.bacc as bacc
import concourse.bass as bass
import concourse.mybir as mybir
import concourse.tile as tile
from concourse.bass_utils import run_bass_kernel_spmd
from concourse.masks import make_identity

F = 128  # feature dim (fixed by problem)
NCORES = 8
CH = 32768  # rows addressable per gather call (int16 indices)
# max segment length per reduce group; dma_scatter_add loses colliding
# updates within one call, so each (node, chunk) must stay a single group
# (scatter is issued per chunk region -> unique rows per call)
DCAP = 128
# exact degree buckets: within-group padding would gather real table rows
# into a real node's sum (there is no per-edge weight to zero them out)
DBUCKETS = list(range(1, DCAP + 1))
_BUCKET_LUT = np.zeros(DCAP + 1, dtype=np.int64)
for _d in range(1, DCAP + 1):
    _BUCKET_LUT[_d] = min(b for b in DBUCKETS if b >= _d)

f32 = mybir.dt.float32
f16 = mybir.dt.float16
i16 = mybir.dt.int16


def _wrap16(flat, ncols):
    """Wrap a flat int16 index stream into the [128, ncols] layout the Q7
    gather/scatter ucode expects: slot i at [i % 16, i // 16], replicated
    across the eight 16-partition core groups."""
    n = flat.shape[0]
    assert n % 16 == 0 and n // 16 <= ncols
    a = np.full((16, ncols), -1, dtype=np.int16)
    a[:, : n // 16] = flat.reshape(n // 16, 16).T
    return np.tile(a, (8, 1))


class Schedule:
    """Uniform (cross-core) aggregation schedule + per-core index data."""

    def __init__(self, n_nodes, tile_nodes):
        self.N = n_nodes
        self.NS = n_nodes // NCORES
        self.NT = -(-self.NS // 128)  # node row-tiles per core
        self.NSP = self.NT * 128  # padded shard rows
        self.NROWS = self.NSP * NCORES  # gather table rows
        self.NCH = -(-self.NROWS // CH)  # chunks
        self.TSN = tile_nodes
        self.n_tiles = -(-self.NSP // tile_nodes)
        # filled by build():
        self.runs = None  # [t][k] -> list of (bucket, m)
        self.S = None  # [t][k] -> padded gather slots
        self.gcol0 = None  # [t][k] -> gidx col offset
        self.P = None  # [t][k] -> partials in chunk region (scatter num_idxs)
        self.R0 = None  # [t][k] -> arena col offset of chunk region (x128)
        self.AC = None  # [t] -> arena cols (sum of padded regions)
        self.scol0 = None  # [t][k] -> sidx col offset
        self.GCOLS = 0
        self.SCOLS = 0
        self.gidx = None  # per-core [128, GCOLS] int16
        self.sidx = None  # per-core [128, SCOLS] int16


def build_schedule(src, dst, n_nodes, tile_nodes=2048):
    """Host preprocessing: group edges by (dst-core, dst-node, src-chunk),
    bucket group sizes, take the max group count over cores per
    (tile, chunk, bucket) as the shared quota, and serialize per-core
    gather/scatter index streams."""
    sch = Schedule(n_nodes, tile_nodes)
    NS, NSP, NCH, TSN = sch.NS, sch.NSP, sch.NCH, sch.TSN
    n_tiles = sch.n_tiles

    src = np.asarray(src, dtype=np.int64)
    dst = np.asarray(dst, dtype=np.int64)
    src_row = (src // NS) * NSP + (src % NS)  # table row (padded layout)
    core = dst // NS
    nloc = dst % NS
    chunk = src_row // CH

    # per-core grouped edges
    per_core = []  # (nloc_g, chunk_g, len_g, tile_g, bucket_g, srcs_sorted)
    for c in range(NCORES):
        m = core == c
        nl, ck, sr = nloc[m], chunk[m], src_row[m]
        order = np.lexsort((sr, ck, nl))
        nl, ck, sr = nl[order], ck[order], sr[order]
        if nl.size == 0:
            per_core.append(
                (np.zeros(0, np.int64),) * 5 + (np.zeros(0, np.int64),)
            )
            continue
        newg = np.empty(nl.size, dtype=bool)
        newg[0] = True
        newg[1:] = (nl[1:] != nl[:-1]) | (ck[1:] != ck[:-1])
        starts = np.flatnonzero(newg)
        lens = np.diff(np.append(starts, nl.size))
        g_nl, g_ck = nl[starts], ck[starts]
        # split groups longer than DCAP
        big = lens > DCAP
        if big.any():
            ns, nl2, nc2, nln = [], [], [], []
            for s, l, a, b in zip(starts[big], lens[big], g_nl[big], g_ck[big]):
                off = 0
                while off < l:
                    take = min(DCAP, l - off)
                    ns.append(s + off)
                    nln.append(take)
                    nl2.append(a)
                    nc2.append(b)
                    off += take
            starts = np.concatenate([starts[~big], np.array(ns, np.int64)])
            lens = np.concatenate([lens[~big], np.array(nln, np.int64)])
            g_nl = np.concatenate([g_nl[~big], np.array(nl2, np.int64)])
            g_ck = np.concatenate([g_ck[~big], np.array(nc2, np.int64)])
        g_tile = g_nl // TSN
        g_bucket = _BUCKET_LUT[lens]
        per_core.append((g_nl, g_ck, lens, g_tile, g_bucket, sr, starts))

    # quotas: max #groups over cores per (tile, chunk, bucket)
    quota = {}
    for c in range(NCORES):
        g_nl, g_ck, lens, g_tile, g_bucket, sr, starts = per_core[c]
        key = (g_tile * NCH + g_ck) * (DCAP + 1) + g_bucket
        uk, cnt = np.unique(key, return_counts=True)
        for k, n in zip(uk, cnt):
            quota[int(k)] = max(quota.get(int(k), 0), int(n))

    runs = [[[] for _ in range(NCH)] for _ in range(n_tiles)]
    for k, n in sorted(quota.items()):
        b = k % (DCAP + 1)
        tk = k // (DCAP + 1)
        t, ck = tk // NCH, tk % NCH
        runs[t][ck].append((int(b), int(n)))
    for t in range(n_tiles):
        for ck in range(NCH):
            runs[t][ck].sort(key=lambda x: -x[0])  # big buckets first

    S = [[0] * NCH for _ in range(n_tiles)]
    gcol0 = [[0] * NCH for _ in range(n_tiles)]
    P = [[0] * NCH for _ in range(n_tiles)]
    R0 = [[0] * NCH for _ in range(n_tiles)]
    AC = [0] * n_tiles
    scol0 = [[0] * NCH for _ in range(n_tiles)]
    gc = 0
    sc = 0
    for t in range(n_tiles):
        ac = 0
        for ck in range(NCH):
            s = sum(b * m for b, m in runs[t][ck])
            s = -(-s // 128) * 128
            S[t][ck] = s
            gcol0[t][ck] = gc
            gc += s // 16
            p = sum(m for _, m in runs[t][ck])
            P[t][ck] = p
            R0[t][ck] = ac
            ac += -(-p // 128) * 128
            scol0[t][ck] = sc
            sc += -(-p // 16)
        AC[t] = ac
    sch.runs, sch.S, sch.gcol0 = runs, S, gcol0
    sch.P, sch.R0, sch.AC, sch.scol0 = P, R0, AC, scol0
    sch.GCOLS, sch.SCOLS = max(gc, 16), max(sc, 16)

    # serialize per-core index streams
    sch.gidx, sch.sidx = [], []
    for c in range(NCORES):
        g_nl, g_ck, lens, g_tile, g_bucket, sr, starts = per_core[c]
        key = ((g_tile * NCH + g_ck) * (DCAP + 1) + g_bucket).astype(np.int64)
        order = np.argsort(key, kind="stable")
        # group boundaries per (t, ck, b) in sorted order
        k_sorted = key[order]
        gi = np.zeros((128, sch.GCOLS), dtype=np.int16)
        si = np.full((128, sch.SCOLS), -1, dtype=np.int16)
        for t in range(n_tiles):
            for ck in range(NCH):
                flat = []
                sflat = []
                for b, mq in runs[t][ck]:
                    kk = (t * NCH + ck) * (DCAP + 1) + b
                    lo = np.searchsorted(k_sorted, kk, "left")
                    hi = np.searchsorted(k_sorted, kk, "right")
                    mem = order[lo:hi]
                    mreal = hi - lo
                    assert mreal <= mq
                    mat = np.zeros((mq, b), dtype=np.int16)
                    if mreal:
                        l_g = lens[mem]
                        rows = np.repeat(np.arange(mreal), l_g)
                        cols = np.arange(l_g.sum()) - np.repeat(
                            np.cumsum(l_g) - l_g, l_g
                        )
                        take = np.repeat(starts[mem], l_g) + cols
                        mat[rows, cols] = (sr[take] - ck * CH).astype(np.int16)
                    flat.append(mat.reshape(-1))
                    srow = np.full(mq, sch.NSP, dtype=np.int16)  # dump row
                    if mreal:
                        srow[:mreal] = g_nl[mem].astype(np.int16)
                    sflat.append(srow)
                flat = (
                    np.concatenate(flat) if flat else np.zeros(0, np.int16)
                )
                pad = S[t][ck] - flat.shape[0]
                flat = np.concatenate([flat, np.zeros(pad, np.int16)])
                nc_ = S[t][ck] // 16
                if nc_:
                    gi[:, gcol0[t][ck] : gcol0[t][ck] + nc_] = _wrap16(
                        flat, nc_
                    )
                sflat = (
                    np.concatenate(sflat) if sflat else np.zeros(0, np.int16)
                )
                ncs = -(-P[t][ck] // 16)
                pad = ncs * 16 - sflat.shape[0]
                sflat = np.concatenate([sflat, np.full(pad, -1, np.int16)])
                if ncs:
                    si[:, scol0[t][ck] : scol0[t][ck] + ncs] = _wrap16(
                        sflat, ncs
                    )
        sch.gidx.append(gi)
        sch.sidx.append(si)
    return sch


def build_bass(sch, debug_dump=False, repeat=1, fake_cc=False, ablate=()):
    """Build the single SPMD Bass program.

    fake_cc=True builds a single-core variant with the AllGather replaced
    by an equivalent-dependency local DMA — for TimelineSim cost analysis
    only (TimelineSim is single-core and can't model collectives).
    """
    NT, NSP, NROWS, NCH = sch.NT, sch.NSP, sch.NROWS, sch.NCH
    n_tiles = sch.n_tiles
    NSPD = NSP + 128  # accumulator rows (incl. dump row at NSP)
    SMAX = max(max(r) for r in sch.S)
    # partial row-blocks per chunk region
    RMAXB = max(
        -(-p // 128) for row in sch.P for p in row if p
    )
    SXMAX = max(-(-p // 16) for row in sch.P for p in row if p)

    nc = bacc.Bacc(
        "TRN2",
        target_bir_lowering=False,
        debug=False,
        enable_asserts=False,
        num_devices=1 if fake_cc else NCORES,
        num_swdge_queues=2,
    )

    # I/O
    x_in = nc.dram_tensor("x", [NSP, F], f32, kind="ExternalInput").ap()
    gidx_in = nc.dram_tensor(
        "gidx", [128, sch.GCOLS], i16, kind="ExternalInput"
    ).ap()
    sidx_in = nc.dram_tensor(
        "sidx", [128, sch.SCOLS], i16, kind="ExternalInput"
    ).ap()
    dinv_in = nc.dram_tensor("dinv", [128, NT], f32, kind="ExternalInput").ap()
    w1_in = nc.dram_tensor("w1", [F, F], f32, kind="ExternalInput").ap()
    w2_in = nc.dram_tensor("w2", [F, F], f32, kind="ExternalInput").ap()
    wh_in = nc.dram_tensor("wh", [F, 32], f32, kind="ExternalInput").ap()
    b1_in = nc.dram_tensor("b1r", [128, F], f32, kind="ExternalInput").ap()
    b2_in = nc.dram_tensor("b2r", [128, F], f32, kind="ExternalInput").ap()
    bh_in = nc.dram_tensor("bhr", [128, 32], f32, kind="ExternalInput").ap()
    zeros_in = nc.dram_tensor(
        "zeros", [NSPD, F], f32, kind="ExternalInput"
    ).ap()
    out_dram = nc.dram_tensor(
        "out", [NSP, 32], f32, kind="ExternalOutput"
    ).ap()

    # internal DRAM
    z_loc = [
        nc.dram_tensor(f"z_loc{i}", [NSP, F], f16).ap() for i in range(2)
    ]
    z_tab = [
        nc.dram_tensor(
            f"z_tab{i}",
            [NROWS, F],
            f16,
            addr_space="Local" if fake_cc else "Shared",
        ).ap()
        for i in range(2)
    ]
    acc = [nc.dram_tensor(f"acc{i}", [NSPD, F], f32).ap() for i in range(2)]

    rg = [list(range(NCORES))]

    with tile.TileContext(nc) as tc:
        with (
            tc.tile_pool(name="const", bufs=1) as constp,
            tc.tile_pool(name="big", bufs=1) as bigp,
            tc.tile_pool(name="msg", bufs=int(os.environ.get("KMSGB", 3))) as msgp,
            tc.tile_pool(
                name="arena", bufs=int(os.environ.get("KARB", 3))
            ) as arenap,
            tc.tile_pool(
                name="prow", bufs=int(os.environ.get("KPRB", 3))
            ) as prowp,
            tc.tile_pool(name="gix", bufs=int(os.environ.get("KGIXB", 4))) as gixp,
            tc.tile_pool(name="row", bufs=int(os.environ.get("KROWB", 2))) as rowp,
            tc.tile_pool(name="psum", bufs=3, space="PSUM") as psump,
            tc.tile_pool(name="psumh", bufs=2, space="PSUM") as psumhp,
            tc.tile_pool(name="psz", bufs=2, space="PSUM") as pszp,
        ):
            ident = constp.tile([128, 128], f32, tag="ident")
            make_identity(nc, ident[:])
            w1 = constp.tile([F, F], f32, tag="w1")
            nc.sync.dma_start(w1[:], w1_in)
            w2 = constp.tile([F, F], f32, tag="w2")
            nc.sync.dma_start(w2[:], w2_in)
            wh = constp.tile([F, 32], f32, tag="wh")
            nc.sync.dma_start(wh[:], wh_in)
            b1r = constp.tile([128, F], f32, tag="b1r")
            nc.sync.dma_start(b1r[:], b1_in)
            b2r = constp.tile([128, F], f32, tag="b2r")
            nc.sync.dma_start(b2r[:], b2_in)
            bhr = constp.tile([128, 32], f32, tag="bhr")
            nc.sync.dma_start(bhr[:], bh_in)
            dinv = constp.tile([128, NT], f32, tag="dinv")
            nc.sync.dma_start(dinv[:], dinv_in)

            # persistent [F, NSP] transposed activations (xT, then h1T, h2T)
            hT = bigp.tile([128, NSP], f32, tag="hT")

            def load_xT():
                # batch 8 row-tiles per DMA: small-DMA fixed cost dominates
                # the HWDGE ring during startup
                xv = x_in.rearrange("(a p) f -> p a f", p=128)
                for q in range(0, NT, 4):
                    nb = min(4, NT - q)
                    r = rowp.tile([128, 4, F], f32, tag="ldrow")
                    nc.sync.dma_start(r[:, :nb, :], xv[:, q : q + nb, :])
                    for j in range(nb):
                        ps = psump.tile([128, 128], f32, tag="tp")
                        nc.tensor.transpose(ps[:], r[:, j, :], ident[:])
                        nc.scalar.copy(
                            hT[:, (q + j) * 128 : (q + j + 1) * 128], ps[:]
                        )

            def z_phase(li, w):
                """z = dinv * (h @ W) from hT; write fp16 rows to z_loc."""
                zv = z_loc[li].rearrange("(a p) f -> p a f", p=128)
                for q in range(0, NT, 4):
                    nb = min(4, NT - q)
                    zr = rowp.tile([128, 4, F], f16, tag="zrow")
                    for j in range(nb):
                        ps = pszp.tile([128, F], f32, tag="zp")
                        nc.tensor.matmul(
                            ps[:],
                            lhsT=hT[:, (q + j) * 128 : (q + j + 1) * 128],
                            rhs=w[:],
                            start=True,
                            stop=True,
                        )
                        nc.scalar.activation(
                            zr[:, j, :],
                            ps[:],
                            mybir.ActivationFunctionType.Copy,
                            scale=dinv[:, q + j : q + j + 1],
                        )
                    nc.sync.dma_start(zv[:, q : q + nb, :], zr[:, :nb, :])
                # init accumulator with zeros, gather table via AllGather
                nc.sync.dma_start(acc[li][:, :], zeros_in)
                if fake_cc:
                    nc.sync.dma_start(z_tab[li][:NSP, :], z_loc[li][:, :])
                else:
                    nc.gpsimd.collective_compute(
                        "AllGather",
                        mybir.AluOpType.bypass,
                        replica_groups=rg,
                        ins=[z_loc[li][:, :]],
                        outs=[z_tab[li][:, :]],
                    )

            def finish_rows(li, brep, j0, j1):
                """h = relu(dinv*(acc+z)+b) for node row-tiles [j0, j1)."""
                av = acc[li].rearrange("(a p) f -> p a f", p=128)
                zv = z_loc[li].rearrange("(a p) f -> p a f", p=128)
                for q in range(j0, j1, 4):
                    nb = min(4, j1 - q)
                    at8 = rowp.tile([128, 4, F], f32, tag="accrow")
                    nc.sync.dma_start(at8[:, :nb, :], av[:, q : q + nb, :])
                    zt8 = rowp.tile([128, 4, F], f16, tag="zrd")
                    nc.sync.dma_start(zt8[:, :nb, :], zv[:, q : q + nb, :])
                    for i in range(nb):
                        j = q + i
                        at = at8[:, i, :]
                        nc.vector.tensor_tensor(
                            out=at, in0=at, in1=zt8[:, i, :],
                            op=mybir.AluOpType.add,
                        )
                        nc.vector.tensor_scalar_mul(
                            at, at, dinv[:, j : j + 1]
                        )
                        nc.vector.tensor_tensor(
                            out=at, in0=at, in1=brep[:],
                            op=mybir.AluOpType.add,
                        )
                        hr = rowp.tile([128, F], f32, tag="hrow")
                        nc.scalar.activation(
                            hr[:], at, mybir.ActivationFunctionType.Relu
                        )
                        ps = psumhp.tile([128, 128], f32, tag="tph")
                        nc.tensor.transpose(ps[:], hr[:], ident[:])
                        nc.scalar.copy(
                            hT[:, j * 128 : (j + 1) * 128], ps[:]
                        )

            def agg_phase(li, brep):
                for t in range(n_tiles):
                    # chunk regions of one tile hit the same acc rows, so
                    # their scatter-adds are chained (dma_scatter_add loses
                    # colliding in-flight CCE RMWs); different tiles target
                    # disjoint rows and overlap freely
                    prev_sc = None
                    for ck in range(NCH):
                        S = sch.S[t][ck]
                        P = sch.P[t][ck]
                        if S == 0:
                            continue
                        gx = gixp.tile([128, SMAX // 16], i16, tag="gx")
                        c0 = sch.gcol0[t][ck]
                        nc.sync.dma_start(
                            gx[:, : S // 16], gidx_in[:, c0 : c0 + S // 16]
                        )
                        msg = msgp.tile([128, SMAX], f16, tag="msg")
                        rows = min(CH, NROWS - ck * CH)
                        if "gather" in ablate:
                            continue
                        nc.gpsimd.dma_gather(
                            out_ap=msg[:, :S].rearrange(
                                "p (a s) -> p a s", a=1
                            ),
                            in_ap=z_tab[li][ck * CH : ck * CH + rows, :],
                            idxs_ap=gx[:, : S // 16],
                            num_idxs=S,
                            num_idxs_reg=S,
                            elem_size=F,
                            transpose=True,
                            single_packet=False,
                            queue_num=(t * NCH + ck) % 2,
                        )
                        pblk = -(-P // 128)
                        arena = arenap.tile([128, RMAXB * 128], f32, tag="ar")
                        if "reduce" not in ablate:
                            off = 0
                            po = 0
                            for b, m in sch.runs[t][ck]:
                                nc.vector.tensor_reduce(
                                    out=arena[:, po : po + m],
                                    in_=msg[:, off : off + m * b].rearrange(
                                        "p (m b) -> p m b", b=b
                                    ),
                                    axis=mybir.AxisListType.X,
                                    op=mybir.AluOpType.add,
                                )
                                po += m
                                off += m * b
                        if "txp" in ablate:
                            continue
                        # transpose partials to rows, scatter into acc;
                        # 4 transposes share one psum bank so one ACT copy
                        # moves [128, 512] per trip
                        pr = prowp.tile([128, RMAXB, 128], f32, tag="pr")
                        for bq in range(0, pblk, 4):
                            nb = min(4, pblk - bq)
                            ps = psump.tile([128, 512], f32, tag="tp")
                            for j in range(nb):
                                nc.tensor.transpose(
                                    ps[:, j * 128 : (j + 1) * 128],
                                    arena[
                                        :,
                                        (bq + j) * 128 : (bq + j + 1) * 128,
                                    ],
                                    ident[:],
                                )
                            nc.scalar.copy(
                                pr[:, bq : bq + nb, :], ps[:, : nb * 128]
                            )
                        if "scatter" in ablate:
                            continue
                        ncs = -(-P // 16)
                        sx = gixp.tile([128, SXMAX], i16, tag="sx")
                        s0 = sch.scol0[t][ck]
                        nc.sync.dma_start(
                            sx[:, :ncs], sidx_in[:, s0 : s0 + ncs]
                        )
                        sc_inst = nc.gpsimd.dma_scatter_add(
                            out_ap=acc[li][:, :],
                            in_ap=pr[:, :pblk, :],
                            idxs_ap=sx[:, :ncs],
                            num_idxs=P,
                            num_idxs_reg=P,
                            elem_size=F,
                            single_packet=False,
                            queue_num=(t * NCH + ck) % 2,
                        )
                        if prev_sc is not None and "chain" not in ablate:
                            tile.add_dep_helper(
                                sc_inst.ins,
                                prev_sc.ins,
                                sync=True,
                                reason="serialize same-tile acc RMW",
                            )
                        prev_sc = sc_inst
                    # finish this tile's node rows as soon as its scatters
                    # land, overlapping with later tiles' gathers
                    if "finish" not in ablate:
                        j0 = t * sch.TSN // 128
                        j1 = min((t + 1) * sch.TSN, NSP) // 128
                        finish_rows(li, brep, j0, j1)

            def heads():
                ov = out_dram.rearrange("(a p) f -> p a f", p=128)
                for q in range(0, NT, 4):
                    nb = min(4, NT - q)
                    ot = rowp.tile([128, 4, 32], f32, tag="orow")
                    for j in range(nb):
                        ps = pszp.tile([128, 32], f32, tag="zp")
                        nc.tensor.matmul(
                            ps[:],
                            lhsT=hT[:, (q + j) * 128 : (q + j + 1) * 128],
                            rhs=wh[:],
                            start=True,
                            stop=True,
                        )
                        nc.vector.tensor_tensor(
                            out=ot[:, j, :], in0=ps[:], in1=bhr[:],
                            op=mybir.AluOpType.add,
                        )
                    nc.sync.dma_start(ov[:, q : q + nb, :], ot[:, :nb, :])

            for _rep in range(repeat):
                load_xT()
                z_phase(0, w1)
                agg_phase(0, b1r)
                z_phase(1, w2)
                agg_phase(1, b2r)
                heads()
            if debug_dump:
                dz = nc.dram_tensor(
                    "dbg_z", [NSP, F], f16, kind="ExternalOutput"
                ).ap()
                dt_ = nc.dram_tensor(
                    "dbg_tab", [NROWS, F], f16, kind="ExternalOutput"
                ).ap()
                da = nc.dram_tensor(
                    "dbg_acc", [NSPD, F], f32, kind="ExternalOutput"
                ).ap()
                nc.sync.dma_start(dz, z_loc[0][:, :])
                nc.sync.dma_start(dt_, z_tab[0][:, :])
                nc.sync.dma_start(da, acc[0][:, :])

    nc.compile()
    return nc


def host_preprocess(inputs, n_nodes, tile_nodes=2048):
    x = np.asarray(inputs["x"], dtype=np.float32)
    ei = np.asarray(inputs["edge_index"])
    src, dst = ei[0].astype(np.int64), ei[1].astype(np.int64)

    deg = (np.bincount(dst, minlength=n_nodes) + 1).astype(np.float32)
    dinv = (1.0 / np.sqrt(deg)).astype(np.float32)

    sch = build_schedule(src, dst, n_nodes, tile_nodes)
    NS, NSP, NT = sch.NS, sch.NSP, sch.NT

    wh = np.concatenate(
        [np.asarray(inputs["Wm"], np.float32), np.asarray(inputs["Ws"], np.float32)],
        axis=1,
    )
    bh = np.concatenate(
        [np.asarray(inputs["bm"], np.float32), np.asarray(inputs["bs"], np.float32)]
    )
    b1 = np.asarray(inputs["b1"], np.float32)
    b2 = np.asarray(inputs["b2"], np.float32)

    in_maps = []
    for c in range(NCORES):
        xs = np.zeros((NSP, F), np.float32)
        xs[:NS] = x[c * NS : (c + 1) * NS]
        dv = np.ones(NSP, np.float32)
        dv[:NS] = dinv[c * NS : (c + 1) * NS]
        in_maps.append(
            {
                "x": xs,
                "gidx": sch.gidx[c],
                "sidx": sch.sidx[c],
                "dinv": dv.reshape(NT, 128).T.copy(),
                "w1": np.asarray(inputs["W1"], np.float32),
                "w2": np.asarray(inputs["W2"], np.float32),
                "wh": wh,
                "b1r": np.tile(b1[None, :], (128, 1)),
                "b2r": np.tile(b2[None, :], (128, 1)),
                "bhr": np.tile(bh[None, :], (128, 1)),
                "zeros": np.zeros((NSP + 128, F), np.float32),
            }
        )
    return sch, in_maps


def run_gcn(inputs, n_nodes, tile_nodes=2048, trace=False, repeat=1, **run_kwargs):
    sch, in_maps = host_preprocess(inputs, n_nodes, tile_nodes)
    nc = build_bass(sch, repeat=repeat)
    res = run_bass_kernel_spmd(
        nc, in_maps, list(range(NCORES)), trace=trace, **run_kwargs
    )
    NS = sch.NS
    outs = [np.asarray(res.results[c]["out"])[:NS] for c in range(NCORES)]
    full = np.concatenate(outs, axis=0)
    mean = np.ascontiguousarray(full[:, :16])
    logstd = np.ascontiguousarray(full[:, 16:])
    return (mean, logstd), res


def kernel(**inputs):
    (mean, logstd), _ = run_gcn(inputs, n_nodes=100000)
    return mean, logstd
